# revision 65
# baseline (speedup 1.0000x reference)
"""Attention-score softmax kernel for Trainium2 (8 NeuronCores, SPMD).

reference:
    energies = history @ W.T + b          # [seq, hid]
    scores   = out_state @ energies.T     # [state, seq]
    out      = softmax(scores, axis=-1)

Key algebra: scores = out_state @ W @ history.T + (out_state @ b) 1^T.
The bias term is constant along each row, so it drops out of the row
softmax.  We therefore compute  softmax(out_state @ W @ history.T)
as two chained matmuls (25.8 GMAC total instead of 68.7 GFLOP naive):
    T      = out_state @ W                 # [state, hid]
    scores = T @ history.T                 # [state, seq]

Sharding: rows of out_state (and of the output) are split 8 ways;
W and history are replicated.  Per core:
    MM1: T_c.T = W.T @ S_c.T   -> lhsT = W[e,h] (natural), rhs = S_c.T [e,i]
    MM2: scores_c = T_c @ history.T -> lhsT = T_c.T [h,i], rhs = hist.T [h,j]
    online row softmax: per-slab exp(x - m_s) straight out of PSUM on the
    scalar engine (fused row-sum via accum_out), then one final per-slab
    rescale by exp(m_s - M)/Z fused into the output pass.

All matmul inputs are fp16 (host-cast): full PE rate (1 cyc/row), and the
input-rounding noise on the logits is ~0.01 abs which keeps the softmax
L2 rel err ~3e-3 (validated numerically against the fp32 reference).
Accumulation is fp32 in PSUM; softmax runs in fp32.
"""

import contextlib

import numpy as np

STATE, SEQ, HID, NCORES = 2048, 4096, 2048, 8
IS = STATE // NCORES   # 256 out_state rows per core
NE = HID // 128        # 16 contraction chunks (e) for MM1
NH = HID // 128        # 16 h tiles (contraction for MM2)
NSLAB = SEQ // 512     # 8 j slabs of 512
NI = IS // 128         # 2 output partition tiles

TRACE = False
_CACHE: dict = {}


def _build():
    import concourse.bacc as bacc
    import concourse.mybir as mybir
    import concourse.tile as tile

    f16, f32 = mybir.dt.float16, mybir.dt.float32
    X = mybir.AxisListType.X
    Alu = mybir.AluOpType
    Exp = mybir.ActivationFunctionType.Exp

    nc = bacc.Bacc("TRN2", target_bir_lowering=False, debug=False)
    s_t = nc.dram_tensor("s_t", [HID, IS], f16, kind="ExternalInput")
    w = nc.dram_tensor("w", [HID, HID], f16, kind="ExternalInput")
    hist_t = nc.dram_tensor("hist_t", [HID, SEQ], f16, kind="ExternalInput")
    # fp16 output staging: probs are in [0,1]; the host upcasts to fp32.
    # Halves the output-DMA tail (2.1MB/core instead of 4.2MB).
    out = nc.dram_tensor("out", [IS, SEQ], f16, kind="ExternalOutput")

    with tile.TileContext(nc) as tc:
        with (
            tc.tile_pool(name="res", bufs=1) as res,
            tc.tile_pool(name="wstream", bufs=8) as wstream,
            tc.tile_pool(name="hstream", bufs=4) as hstream,
            tc.tile_pool(name="psum", bufs=8, space="PSUM") as psum,
        ):
            # ---- input DMA (issue order ~ priority order) ----
            wch = [wstream.tile([128, HID], f16, tag="wch", name=f"wch{e}")
                   for e in range(NE)]
            hist_tiles = [
                hstream.tile([128, NH, 512], f16, tag="hist", name=f"hist{s}")
                for s in range(NSLAB)
            ]

            def hist_src(s):
                return hist_t[:, s * 512:(s + 1) * 512].rearrange(
                    "(ht p) j -> p ht j", p=128
                )

            # split the first W chunk + S_T so the first matmul starts sooner;
            # S_T pieces are staged between W chunks so W never starves MM1.
            st = res.tile([128, NE, IS], f16, tag="st", name="st")
            st_src = s_t[:, :].rearrange("(e p) i -> p e i", p=128)
            nc.sync.dma_start(wch[0][:, 0:1024], w[0:128, 0:1024])
            nc.sync.dma_start(st[:, 0:1, :], st_src[:, 0:1, :])
            nc.sync.dma_start(wch[0][:, 1024:2048], w[0:128, 1024:2048])
            nc.sync.dma_start(st[:, 1:2, :], st_src[:, 1:2, :])
            # one S_T piece after each W chunk: both streams stay just
            # ahead of the PE's (w[e], st[e]) consumption cadence.
            for e in range(1, NE):
                nc.sync.dma_start(wch[e][:], w[e * 128:(e + 1) * 128, :])
                if e + 1 < NE:
                    nc.sync.dma_start(st[:, e + 1:e + 2, :],
                                      st_src[:, e + 1:e + 2, :])
            for s in range(NSLAB):
                nc.sync.dma_start(hist_tiles[s][:, 0:NH // 2, :],
                                  hist_src(s)[:, 0:NH // 2, :])
                nc.sync.dma_start(hist_tiles[s][:, NH // 2:, :],
                                  hist_src(s)[:, NH // 2:, :])

            # ---- PE warm-up: the HAM clock gate starts cold (1.2 GHz) and
            # needs ~3.4us of sustained activity to reach 2.4 GHz.  Dummy
            # matmuls on a zeroed tile fill the initial DMA wait; neutral in
            # the cost model (MM1 is DMA-paced there) but a free win if real
            # HBM streams faster than the model's 0.83x derate (measured
            # silicon reaches ~97% at large transfers, making MM1 PE-paced).
            warm = res.tile([128, 128], f16, tag="warm", name="warm")
            nc.vector.memset(warm[:], 0.0)
            pwarm = psum.tile([128, 512], f32, tag="ps", name="pwarm")
            NWARM = 26
            for d in range(NWARM):
                nc.tensor.matmul(
                    pwarm[:, 0:128], warm[:], warm[:],
                    start=(d == 0), stop=(d == NWARM - 1),
                )

            # ---- MM1: T.T[h, i] = sum_e W[e,h] * S[i,e] ----
            # psum tile k holds h-tiles (2k, 2k+1) side by side: [128, 512]
            ps1 = [psum.tile([128, 512], f32, tag="ps", name=f"ps1_{k}")
                   for k in range(NH // 2)]
            for e in range(NE):
                for ht in range(NH):
                    col = (ht % 2) * IS
                    nc.tensor.matmul(
                        ps1[ht // 2][:, col:col + IS],
                        wch[e][:, ht * 128:(ht + 1) * 128],  # lhsT [e,h]
                        st[:, e, :],                         # rhs  [e,i]
                        # start clears the WHOLE bank: only the bank's
                        # first-ever matmul may set it; the odd region's
                        # first write overwrites (has_written bit clear).
                        start=(e == 0 and ht % 2 == 0),
                        stop=(e == NE - 1),
                    )
            # drain to fp16 SBUF (T.T resident)
            tt = []
            for k in range(NH // 2):
                t = res.tile([128, 512], f16, tag=f"tt{k}", name=f"tt{k}")
                if k % 2 == 0:
                    nc.vector.tensor_copy(t[:], ps1[k][:])
                else:
                    nc.scalar.copy(t[:], ps1[k][:])
                tt.append(t)

            # ---- MM2 + online softmax (per-slab max, end corrections) ----
            probs, negq, sums = [], [], []
            for i in range(NI):
                probs.append(res.tile([128, SEQ], f32, tag=f"probs{i}", name=f"probs{i}"))
                negq.append(res.tile([128, NSLAB], f32, tag=f"negq{i}", name=f"negq{i}"))
                sums.append(res.tile([128, NSLAB], f32, tag=f"sums{i}", name=f"sums{i}"))

            for s in range(NSLAB):
                for i in range(NI):
                    p2 = psum.tile([128, 512], f32, tag="ps", name=f"ps2_{s}_{i}")
                    for h in range(NH):
                        col = (h % 2) * IS + i * 128
                        nc.tensor.matmul(
                            p2[:],
                            tt[h // 2][:, col:col + 128],   # lhsT [h,i]
                            hist_tiles[s][:, h, :],         # rhs  [h,j]
                            start=(h == 0),
                            stop=(h == NH - 1),
                        )
                    # DVE takes the (negated) slab max straight from PSUM,
                    # then ACT exps PSUM->SBUF with the row sum fused via
                    # accum_out; the exp read frees the bank.
                    sl = slice(s * 512, (s + 1) * 512)
                    prio = (tc.high_priority() if s == NSLAB - 1
                            else contextlib.nullcontext())
                    with prio:
                        nc.vector.reduce_max(negq[i][:, s:s + 1], p2[:],
                                             axis=X, negate=True)
                        nc.scalar.activation(
                            probs[i][:, sl],
                            p2[:],
                            Exp,
                            bias=negq[i][:, s:s + 1],
                            scale=1.0,
                            accum_out=sums[i][:, s:s + 1],
                        )

            # ---- softmax finish: corr_s = exp(m_s - M), Z = sum corr*sums,
            #      out = probs * corr_s / Z (fp16) ----
            # Both correction chains are emitted before any scale op so the
            # ACT FIFO isn't clogged; i0's scales run during i1's last MMs.
            out16 = [res.tile([128, SEQ], f16, tag=f"out16_{i}", name=f"out16_{i}")
                     for i in range(NI)]
            corr, inv = [], []
            for i in range(NI):
                negM = res.tile([128, 1], f32, tag=f"negM{i}", name=f"negM{i}")
                nc.vector.tensor_reduce(out=negM[:], in_=negq[i][:], axis=X,
                                        op=Alu.min)
                c = res.tile([128, NSLAB], f32, tag=f"corr{i}", name=f"corr{i}")
                # corr = exp(negq * -1 + negM) = exp(m_s - M)
                nc.scalar.activation(c[:], negq[i][:], Exp,
                                     bias=negM[:, 0:1], scale=-1.0)
                zp = res.tile([128, NSLAB], f32, tag=f"zp{i}", name=f"zp{i}")
                z = res.tile([128, 1], f32, tag=f"z{i}", name=f"z{i}")
                nc.vector.tensor_mul(zp[:], sums[i][:], c[:])
                nc.vector.reduce_sum(z[:], zp[:], axis=X)
                iv = res.tile([128, 1], f32, tag=f"inv{i}", name=f"inv{i}")
                nc.vector.reciprocal(iv[:], z[:])
                fi = res.tile([128, NSLAB], f32, tag=f"f{i}", name=f"f{i}")
                nc.vector.tensor_scalar_mul(fi[:], c[:], iv[:, 0:1])
                corr.append(fi)
                inv.append(iv)
            for i in range(NI):
                for s in range(NSLAB):
                    sl = slice(s * 512, (s + 1) * 512)
                    # split scales across ACT and DVE so the tail runs on
                    # both engines in parallel (DVE is ~2x faster per op,
                    # so DVE takes 5 of 8)
                    if s in (1, 3, 5):
                        nc.scalar.mul(out16[i][:, sl], probs[i][:, sl],
                                      mul=corr[i][:, s:s + 1])
                    else:
                        nc.vector.tensor_scalar_mul(out16[i][:, sl],
                                                    probs[i][:, sl],
                                                    corr[i][:, s:s + 1])
                    if s % 2 == 1:
                        # 1024-col chunks, alternating the two HWDGE rings
                        dsl = slice((s - 1) * 512, (s + 1) * 512)
                        eng = nc.sync
                        eng.dma_start(out[i * 128:(i + 1) * 128, dsl],
                                      out16[i][:, dsl])

    nc.finalize()
    return nc


def kernel(**inputs: np.ndarray) -> np.ndarray:
    from concourse.bass_utils import run_bass_kernel_spmd

    out_state = np.asarray(inputs["out_state"], dtype=np.float32)
    history = np.asarray(inputs["history"], dtype=np.float32)
    W = np.asarray(inputs["W"], dtype=np.float32)
    # inputs["b"] intentionally unused: softmax(x + c 1^T) == softmax(x).

    if "nc" not in _CACHE:
        _CACHE["nc"] = _build()
    nc = _CACHE["nc"]

    st16 = out_state.T.astype(np.float16)   # [e, i_global]
    w16 = W.astype(np.float16)              # [e, h] natural layout
    ht16 = history.T.astype(np.float16)     # [h, j]

    in_maps = [
        {
            "s_t": np.ascontiguousarray(st16[:, c * IS:(c + 1) * IS]),
            "w": w16,
            "hist_t": ht16,
        }
        for c in range(NCORES)
    ]
    res = run_bass_kernel_spmd(nc, in_maps, core_ids=list(range(NCORES)), trace=TRACE)
    _CACHE["last_result"] = res
    return np.concatenate(
        [res.results[c]["out"] for c in range(NCORES)], axis=0
    ).astype(np.float32)
